# revision 9
# baseline (speedup 1.0000x reference)
"""Ernie4 MoE (T=2048, H=1024, E=64 top-6, I=512 + shared SwiGLU MLP) on 8 Trainium2 cores.

Strategy: expert parallelism, bf16 compute with an exact-fp32 router.
Each core owns 8 expert slots (host assigns experts to (core, slot) by
routed-count snake order; per-slot FFN capacities are derived from the actual
counts at first call and baked into the program). Per core:
  1. exact-fp32 gate logits (reproduces reference top-6 picks bit-exactly),
     sigmoid scores, top-6 + renormalized combine weights,
  2. per-slot routed token lists via gpsimd sparse_gather (pad slots point at
     the trash row T),
  3. transpose-gathers routed activations (bf16) straight into
     [128, H/128, slots] layout, expert SwiGLU FFN on the PE in bf16
     (fp32 PSUM) over the 16-granular capacity, scales by the fp32 combine
     weight, scatter-ADDs fp32 into outp (row T absorbs pad slots),
  4. shared MLP (SI/8 slice, bf16) written densely to a separate bf16 output.
The host sums outp[0:T] + outsh over the 8 cores in fp64.

Emission order is tuned per engine: PE runs gate -> counts -> shared MLP ->
expert FFNs; the vector queue runs top-k -> compaction masks before the
shared-MLP element-wise tail, so expert 0's gather issues ~60us earlier than
a naive phase ordering.
"""

import numpy as np

T, H, E, K, I, SI = 2048, 1024, 64, 6, 512, 1024
NCORE = 8
EC = E // NCORE          # expert slots per core
CMAX = 384               # hard per-slot capacity ceiling (idx tile sizing)
KC = H // 128            # hidden-dim 128-chunks
ICN = I // 128           # expert-intermediate 128-chunks
TCN = T // 128           # token 128-chunks
SIC = SI // NCORE        # shared-intermediate slice per core
BIG = 1e30

_CACHE = {}


def _rup(x, m):
    return (x + m - 1) // m * m


def _build(caps):
    """caps: per-slot FFN capacities (multiples of 16, <= CMAX)."""
    import concourse.bass as bass
    import concourse.tile as tile
    from concourse import bacc, mybir

    f32 = mybir.dt.float32
    bf16 = mybir.dt.bfloat16
    i32 = mybir.dt.int32
    i16 = mybir.dt.int16
    u32 = mybir.dt.uint32
    AF = mybir.ActivationFunctionType
    OP = mybir.AluOpType
    AX = mybir.AxisListType

    caps128 = [_rup(c, 128) for c in caps]

    nc = bacc.Bacc("TRN2", target_bir_lowering=False, debug=False,
                   enable_asserts=False, num_devices=NCORE)

    xT = nc.dram_tensor("xT", [H, T], f32, kind="ExternalInput").ap()
    xp = nc.dram_tensor("xp", [T + 1, H], bf16, kind="ExternalInput").ap()
    gwT = nc.dram_tensor("gwT", [H, E], f32, kind="ExternalInput").ap()
    biasr = nc.dram_tensor("biasr", [128, E], f32, kind="ExternalInput").ap()
    wg = nc.dram_tensor("wg", [EC, H, I], bf16, kind="ExternalInput").ap()
    wu = nc.dram_tensor("wu", [EC, H, I], bf16, kind="ExternalInput").ap()
    wd = nc.dram_tensor("wd", [EC, I, H], bf16, kind="ExternalInput").ap()
    wsg = nc.dram_tensor("wsg", [H, SIC], bf16, kind="ExternalInput").ap()
    wsu = nc.dram_tensor("wsu", [H, SIC], bf16, kind="ExternalInput").ap()
    wsd = nc.dram_tensor("wsd", [SIC, H], bf16, kind="ExternalInput").ap()
    tokp1 = nc.dram_tensor("tokp1", [16, T // 16], f32, kind="ExternalInput").ap()
    pos24 = nc.dram_tensor("pos24", [16, CMAX // 16], f32, kind="ExternalInput").ap()
    outp = nc.dram_tensor("outp", [T + 1, H], f32, kind="ExternalOutput").ap()
    outsh = nc.dram_tensor("outsh", [T, H], bf16, kind="ExternalOutput").ap()

    cmb_d = nc.dram_tensor("cmb_d", [T + 1, 64], f32, kind="Internal").ap()
    sel_d = nc.dram_tensor("sel_d", [T, EC], f32, kind="Internal").ap()

    with tile.TileContext(nc) as tc:
        with (
            tc.tile_pool(name="consts", bufs=1) as consts,
            tc.tile_pool(name="wpool", bufs=2) as wpool,
            tc.tile_pool(name="etmp", bufs=2) as etmp,
            tc.tile_pool(name="smalls", bufs=1) as smalls,
            tc.tile_pool(name="ps_small", bufs=4, space="PSUM") as ps_s,
            tc.tile_pool(name="ps_big", bufs=2, space="PSUM") as ps_b,
        ):
            # ---- constants + gate-critical loads first on the sync queue ----
            tokp1_sb = consts.tile([16, T // 16], f32)
            nc.sync.dma_start(tokp1_sb[:], tokp1)
            bias_sb = consts.tile([128, E], f32)
            nc.sync.dma_start(bias_sb[:], biasr)
            pos_sb = consts.tile([16, CMAX // 16], f32)
            nc.sync.dma_start(pos_sb[:], pos24)
            ones128 = consts.tile([128, 1], f32)
            nc.vector.memset(ones128[:], 1.0)
            ones16 = consts.tile([1, 16], f32)
            nc.vector.memset(ones16[:], 1.0)

            idxP = smalls.tile([128, EC, CMAX // 16], i16, name="idxP")

            with (
                tc.tile_pool(name="ph1", bufs=2) as ph1,
                tc.tile_pool(name="route", bufs=1) as route,
                tc.tile_pool(name="xpool", bufs=2) as xpool,
            ):
                gwT_sb = ph1.tile([128, KC, E], f32, tag="gwT")
                nc.sync.dma_start(gwT_sb[:], gwT.rearrange("(kc p) e -> p kc e", p=128))

                # xT slabs: even on sync, odd on scalar AHEAD of the weight
                # stream, so gate logits aren't head-of-line blocked.
                xtls = []
                for sl in range(8):
                    xtl = ph1.tile([128, KC, 256], f32, tag="xtl", bufs=2,
                                   name=f"xtl{sl}")
                    eng = nc.sync if sl % 2 == 0 else nc.scalar
                    eng.dma_start(
                        xtl[:], xT.rearrange("(kc p) t -> p kc t", p=128)[:, :, sl * 256:(sl + 1) * 256])
                    xtls.append(xtl)

                wsg_sb = ph1.tile([128, KC, SIC], bf16, tag="wsg")
                nc.sync.dma_start(wsg_sb[:], wsg.rearrange("(kc p) s -> p kc s", p=128))
                wsu_sb = ph1.tile([128, KC, SIC], bf16, tag="wsu")
                nc.sync.dma_start(wsu_sb[:], wsu.rearrange("(kc p) s -> p kc s", p=128))
                wsd_sb = ph1.tile([128, H], bf16, tag="wsd")
                nc.sync.dma_start(wsd_sb[:], wsd)

                # ---- expert weight streaming (scalar HWDGE, after xtl) ----
                wg_sbs, wu_sbs, wd_sbs = [], [], []
                for e in range(EC):
                    wg_sb = wpool.tile([128, KC, I], bf16, tag="wg")
                    nc.scalar.dma_start(wg_sb[:], wg[e].rearrange("(kc p) i -> p kc i", p=128))
                    wu_sb = wpool.tile([128, KC, I], bf16, tag="wu")
                    nc.scalar.dma_start(wu_sb[:], wu[e].rearrange("(kc p) i -> p kc i", p=128))
                    wd_sb = wpool.tile([128, ICN, H], bf16, tag="wd")
                    nc.scalar.dma_start(wd_sb[:], wd[e].rearrange("(ic p) h -> p ic h", p=128))
                    wg_sbs.append(wg_sb); wu_sbs.append(wu_sb); wd_sbs.append(wd_sb)

                scores = route.tile([128, TCN, E], f32, tag="scores")
                a_s = route.tile([128, 8, 256], bf16, tag="a_s")

                # ---- gate logits (exact fp32) + bf16 convert, per slab ----
                xbfs = []
                for sl in range(8):
                    xtl = xtls[sl]
                    for j in range(2):
                        tci = sl * 2 + j
                        pl = ps_s.tile([128, 512], f32, tag="mm_small")
                        for kc in range(KC):
                            nc.tensor.matmul(pl[:, :E], xtl[:, kc, j * 128:(j + 1) * 128],
                                             gwT_sb[:, kc, :], start=(kc == 0), stop=(kc == KC - 1))
                        nc.scalar.activation(scores[:, tci, :], pl[:, :E], AF.Sigmoid)
                    xbf = ph1.tile([128, KC, 256], bf16, tag="xbf", bufs=8,
                                   name=f"xbf{sl}")
                    nc.vector.tensor_copy(xbf[:], xtl[:])
                    xbfs.append(xbf)

                # ---- routing top-k (DVE fp32, heads the vector queue) ----
                work_t = [route.tile([128, TCN, E], f32, tag=f"work{i}", name=f"work{i}")
                          for i in range(2)]
                nc.vector.tensor_tensor(
                    work_t[0][:], scores[:],
                    bias_sb[:, None, :].to_broadcast([128, TCN, E]), op=OP.add)
                wsrc = work_t[0]
                for k in range(K):
                    m = route.tile([128, TCN], f32, tag=f"m{k % 2}")
                    nc.vector.reduce_max(m[:], wsrc[:], axis=AX.X)
                    eq = route.tile([128, TCN, E], f32, tag="eq")
                    nc.vector.tensor_tensor(
                        eq[:], wsrc[:], m[:, :, None].to_broadcast([128, TCN, E]),
                        op=OP.is_equal)
                    wdst = work_t[(k + 1) % 2] if k < K - 1 else work_t[0]
                    nc.vector.scalar_tensor_tensor(
                        wdst[:], eq[:], -BIG, wsrc[:], op0=OP.mult, op1=OP.add)
                    wsrc = wdst
                sel = route.tile([128, TCN, E], f32, tag="eq")
                nc.vector.tensor_scalar(sel[:], wsrc[:], -BIG / 2, None, op0=OP.is_lt)
                selprod = route.tile([128, TCN, E], f32, tag="work1")
                nc.vector.tensor_tensor(selprod[:], scores[:], sel[:], op=OP.mult)
                denom = route.tile([128, TCN], f32, tag="denom")
                nc.vector.tensor_reduce(denom[:], selprod[:], axis=AX.X, op=OP.add)
                rec = route.tile([128, TCN], f32, tag="rec")
                nc.vector.reciprocal(rec[:], denom[:])
                cmb8 = route.tile([128, TCN, EC], f32, tag="cmb8")
                nc.vector.tensor_tensor(
                    cmb8[:], selprod[:, :, 0:EC],
                    rec[:, :, None].to_broadcast([128, TCN, EC]), op=OP.mult)

                # layout roundtrip through DRAM (wrapped 16-row views)
                cmbw = route.tile([128, TCN, 64], f32, tag="work1")
                nc.vector.memset(cmbw[:], 0.0)
                nc.vector.tensor_copy(cmbw[:, :, 0:EC], cmb8[:])
                nc.sync.dma_start(
                    cmb_d[0:T].rearrange("(tc p) e -> p tc e", p=128), cmbw[:])
                zrow = route.tile([1, 64], f32, tag="zrow")
                nc.vector.memset(zrow[:], 0.0)
                nc.sync.dma_start(cmb_d[T:T + 1, :], zrow[:])
                nc.sync.dma_start(sel_d.rearrange("(tc p) e -> p tc e", p=128),
                                  sel[:, :, 0:EC])
                sel16 = route.tile([16, EC, T // 16], f32, tag="sel16")
                nc.sync.dma_start(sel16[:], sel_d.rearrange("(f q) e -> q e f", q=16))

                # per-slot routed counts via PE (emitted before the shared
                # MLP bulk so the PE queue reaches it as soon as sel lands)
                pc = ps_s.tile([1, 512], f32, tag="mm_small", name="pc")
                nc.tensor.matmul(pc[0:1, 0:128], ones128[:],
                                 sel[:, :, 0:EC].rearrange("p t e -> p e t"),
                                 start=True, stop=True)
                counts = route.tile([1, EC], f32, tag="counts")
                nc.vector.tensor_reduce(counts[:], pc[0:1, 0:128].rearrange(
                    "p (e t) -> p e t", e=EC), axis=AX.X, op=OP.add)
                pnf = ps_s.tile([16, 512], f32, tag="mm_small", name="pnf")
                nc.tensor.matmul(pnf[:, 0:EC], ones16[:], counts[:],
                                 start=True, stop=True)
                nf16 = route.tile([16, EC], f32, tag="nf16")
                nc.vector.tensor_copy(nf16[:], pnf[:, 0:EC])

                # masked token values in wrapped layout: sel*(tok+1)-1
                nc.vector.tensor_tensor(
                    sel16[:], sel16[:],
                    tokp1_sb[:, None, :].to_broadcast([16, EC, T // 16]), op=OP.mult)
                nc.vector.tensor_scalar_sub(sel16[:], sel16[:], 1.0)

                # ---- compaction: slot 0 first, then slots 1-7 ----
                nfs = route.tile([1, EC], u32, tag="nfs")
                idxf = route.tile([16, EC, CMAX // 16], f32, tag="idxf")
                nc.vector.memset(idxf[:], 0.0)

                def mask_slots(elo, ehi):
                    sli = (slice(None), slice(elo, ehi), slice(None))
                    shp = [16, ehi - elo, CMAX // 16]
                    keep = route.tile([16, EC, CMAX // 16], f32, tag="keep")
                    nc.vector.tensor_tensor(
                        keep[sli], pos_sb[:, None, :].to_broadcast(shp),
                        nf16[:, elo:ehi, None].to_broadcast(shp), op=OP.is_lt)
                    k32 = route.tile([16, EC, CMAX // 16], i32, tag="k32")
                    nc.vector.tensor_copy(k32[sli], keep[sli])
                    km = route.tile([16, EC, CMAX // 16], i32, tag="km")
                    nc.vector.tensor_scalar_mul(km[sli], k32[sli], -1)
                    bits = route.tile([16, EC, CMAX // 16], i32, tag="bits")
                    nc.vector.tensor_tensor(bits[sli], idxf[sli].bitcast(i32), km[sli],
                                            op=OP.bitwise_and)
                    km1 = route.tile([16, EC, CMAX // 16], f32, tag="km1")
                    nc.vector.tensor_scalar_sub(km1[sli], keep[sli], 1.0)
                    idxnf = route.tile([16, EC, CMAX // 16], f32, tag="idxnf")
                    nc.vector.scalar_tensor_tensor(idxnf[sli], km1[sli], -float(T),
                                                   bits[sli].bitcast(f32),
                                                   op0=OP.mult, op1=OP.add)
                    nc.vector.tensor_copy(idxP[0:16, elo:ehi, :], idxnf[sli])
                    nc.sync.dma_start(idxP[16:32, elo:ehi, :], idxP[0:16, elo:ehi, :])
                    nc.sync.dma_start(idxP[32:64, elo:ehi, :], idxP[0:32, elo:ehi, :])
                    nc.sync.dma_start(idxP[64:128, elo:ehi, :], idxP[0:64, elo:ehi, :])

                xgs, cgs = {}, {}

                def emit_gathers(e):
                    C1 = caps128[e]
                    xg = xpool.tile([128, KC, C1], bf16, tag=f"xe{C1}", name=f"xg{e}")
                    nc.gpsimd.dma_gather(xg[:], xp, idxP[:, e, 0:C1 // 16], C1, C1, H,
                                         transpose=True)
                    cg = xpool.tile([128, 3, 64], f32, tag="cg", name=f"cg{e}")
                    nc.gpsimd.dma_gather(cg[:, 0:C1 // 128, :], cmb_d,
                                         idxP[:, e, 0:C1 // 16], C1, C1, 64)
                    xgs[e], cgs[e] = xg, cg

                nc.gpsimd.sparse_gather(idxf[:, 0, 0:caps128[0] // 16], sel16[:, 0, :],
                                        num_found=nfs[0:1, 0:1])
                mask_slots(0, 1)
                emit_gathers(0)
                for e in range(1, EC):
                    nc.gpsimd.sparse_gather(idxf[:, e, 0:caps128[e] // 16], sel16[:, e, :],
                                            num_found=nfs[0:1, e:e + 1])
                mask_slots(1, EC)
                emit_gathers(1)

                # ---- shared MLP gate/up (bf16; PE fills the routing gap) ----
                for sl in range(8):
                    pg = ps_s.tile([128, 512], f32, tag="mm_small")
                    pu = ps_s.tile([128, 512], f32, tag="mm_small")
                    for kc in range(KC):
                        nc.tensor.matmul(pg[:, :256], wsg_sb[:, kc, :], xbfs[sl][:, kc, :],
                                         start=(kc == 0), stop=(kc == KC - 1))
                    for kc in range(KC):
                        nc.tensor.matmul(pu[:, :256], wsu_sb[:, kc, :], xbfs[sl][:, kc, :],
                                         start=(kc == 0), stop=(kc == KC - 1))
                    sg_t = route.tile([128, 256], f32, tag="sgt")
                    nc.scalar.activation(sg_t[:], pg[:, :256], AF.Sigmoid)
                    gu_t = route.tile([128, 256], f32, tag="gut")
                    nc.vector.tensor_tensor(gu_t[:], sg_t[:], pg[:, :256], op=OP.mult)
                    nc.vector.tensor_tensor(a_s[:, sl, :], gu_t[:], pu[:, :256], op=OP.mult)

                # ---- shared down-proj + dense bf16 write ----
                for tci in range(TCN):
                    sl, j = tci // 2, tci % 2
                    py = ps_b.tile([128, H], f32, tag="mm_big")
                    for nh in range(2):
                        nc.tensor.matmul(py[:, nh * 512:(nh + 1) * 512],
                                         a_s[:, sl, j * 128:(j + 1) * 128],
                                         wsd_sb[:, nh * 512:(nh + 1) * 512],
                                         start=True, stop=True)
                    ysh = route.tile([128, H], bf16, tag="ysh", bufs=2)
                    nc.scalar.activation(ysh[:, 0:512], py[:, 0:512], AF.Copy)
                    nc.vector.tensor_copy(ysh[:, 512:1024], py[:, 512:1024])
                    nc.sync.dma_start(
                        outsh.rearrange("(tc p) h -> p tc h", p=128)[:, tci, :], ysh[:])

                # ---- expert loop (gathers for e+1 prefetched before e's
                # scatter on the soft-DMA queue) ----
                for e in range(EC):
                    C, C1 = caps[e], caps128[e]
                    CCH = (C + 127) // 128
                    if e + 1 < EC:
                        emit_gathers(e + 1)
                    wg_sb, wu_sb, wd_sb = wg_sbs[e], wu_sbs[e], wd_sbs[e]
                    xeT, cg = xgs.pop(e), cgs.pop(e)

                    aT = xpool.tile([128, ICN, CMAX], bf16, tag="aT")
                    for ic in range(ICN):
                        pg = ps_s.tile([128, 512], f32, tag="mm_small")
                        pu = ps_s.tile([128, 512], f32, tag="mm_small")
                        for kc in range(KC):
                            nc.tensor.matmul(pg[:, :C], wg_sb[:, kc, ic * 128:(ic + 1) * 128],
                                             xeT[:, kc, 0:C], start=(kc == 0), stop=(kc == KC - 1))
                        for kc in range(KC):
                            nc.tensor.matmul(pu[:, :C], wu_sb[:, kc, ic * 128:(ic + 1) * 128],
                                             xeT[:, kc, 0:C], start=(kc == 0), stop=(kc == KC - 1))
                        sg_t = etmp.tile([128, CMAX], f32, tag="esilu")
                        nc.scalar.activation(sg_t[:, :C], pg[:, :C], AF.Sigmoid)
                        gu_t = etmp.tile([128, CMAX], f32, tag="egu")
                        nc.vector.tensor_tensor(gu_t[:, :C], sg_t[:, :C], pg[:, :C], op=OP.mult)
                        nc.vector.tensor_tensor(aT[:, ic, 0:C], gu_t[:, :C], pu[:, :C], op=OP.mult)

                    y_sb = xpool.tile([128, 3, H], f32, tag="y")
                    for cc in range(CCH):
                        w = min(128, C - cc * 128)
                        if w < 128:
                            nc.vector.memset(y_sb[:, cc, :], 0.0)
                        py = ps_b.tile([128, H], f32, tag="mm_big")
                        for ic in range(ICN):
                            for nh in range(2):
                                nc.tensor.matmul(py[0:w, nh * 512:(nh + 1) * 512],
                                                 aT[:, ic, cc * 128:cc * 128 + w],
                                                 wd_sb[:, ic, nh * 512:(nh + 1) * 512],
                                                 start=(ic == 0), stop=(ic == ICN - 1))
                        nc.scalar.activation(y_sb[0:w, cc, :], py[0:w, :], AF.Copy,
                                             scale=cg[0:w, cc, e:e + 1])
                    nc.gpsimd.dma_scatter_add(outp, y_sb[:, 0:CCH, :],
                                              idxP[:, e, 0:C // 16], C, C, H)

    nc.compile()
    return nc


def _route_counts(x, gate_w, gate_bias):
    """Host-side routing counts (fp64) for load-balanced expert assignment."""
    logits = x.astype(np.float64) @ gate_w.astype(np.float64).T
    scores = 1.0 / (1.0 + np.exp(-logits))
    idx = np.argsort(-(scores + gate_bias.astype(np.float64)), axis=1)[:, :K]
    return np.bincount(idx.ravel(), minlength=E)


def _assign(counts):
    """Snake expert->(core,slot) assignment + 16-granular slot capacities."""
    order = np.argsort(-counts, kind="stable")
    perm = np.zeros((NCORE, EC), dtype=np.int64)
    caps = []
    for s in range(EC):
        band = order[8 * s:8 * s + 8]
        perm[:, s] = band if s % 2 == 0 else band[::-1]
        c = _rup(int(counts[band].max()) + 8, 16)
        assert c <= CMAX, (s, c)
        caps.append(c)
    return perm, tuple(caps)


def _prep_in_maps(inputs, perm):
    import ml_dtypes
    bf = ml_dtypes.bfloat16
    x = np.ascontiguousarray(inputs["hidden_states"], dtype=np.float32)
    gate_w = np.asarray(inputs["gate_w"], dtype=np.float32)
    gate_bias = np.asarray(inputs["gate_bias"], dtype=np.float32)
    w_gate = np.asarray(inputs["w_gate"], dtype=np.float32)
    w_up = np.asarray(inputs["w_up"], dtype=np.float32)
    w_down = np.asarray(inputs["w_down"], dtype=np.float32)
    ws_gate = np.asarray(inputs["ws_gate"], dtype=np.float32)
    ws_up = np.asarray(inputs["ws_up"], dtype=np.float32)
    ws_down = np.asarray(inputs["ws_down"], dtype=np.float32)

    xTc = np.ascontiguousarray(x.T)
    xbf = np.vstack([x, np.zeros((1, H), np.float32)]).astype(bf)
    tokp1 = (np.arange(16)[:, None] + 16 * np.arange(T // 16)[None, :] + 1).astype(np.float32)
    pos24 = (np.arange(16)[:, None] + 16 * np.arange(CMAX // 16)[None, :]).astype(np.float32)

    in_maps = []
    for c in range(NCORE):
        loc = list(perm[c])
        gorder = loc + [e for e in range(E) if e not in loc]
        in_maps.append({
            "xp": xbf,
            "xT": xTc,
            "gwT": np.ascontiguousarray(gate_w[gorder].T),
            "biasr": np.ascontiguousarray(
                np.broadcast_to(gate_bias[0, gorder], (128, E))).astype(np.float32),
            "wg": np.ascontiguousarray(w_gate[loc]).astype(bf),
            "wu": np.ascontiguousarray(w_up[loc]).astype(bf),
            "wd": np.ascontiguousarray(w_down[loc]).astype(bf),
            "wsg": np.ascontiguousarray(ws_gate[:, c * SIC:(c + 1) * SIC]).astype(bf),
            "wsu": np.ascontiguousarray(ws_up[:, c * SIC:(c + 1) * SIC]).astype(bf),
            "wsd": np.ascontiguousarray(ws_down[c * SIC:(c + 1) * SIC, :]).astype(bf),
            "tokp1": tokp1,
            "pos24": pos24,
        })
    return in_maps


def get_nc(inputs):
    counts = _route_counts(inputs["hidden_states"], inputs["gate_w"], inputs["gate_bias"])
    perm, caps = _assign(counts)
    if ("nc", caps) not in _CACHE:
        _CACHE[("nc", caps)] = _build(caps)
    return _CACHE[("nc", caps)], perm


def kernel(**inputs) -> np.ndarray:
    from concourse import bass_utils
    nc, perm = get_nc(inputs)
    in_maps = _prep_in_maps(inputs, perm)
    res = bass_utils.run_bass_kernel_spmd(nc, in_maps, core_ids=list(range(NCORE)))
    acc = np.zeros((T, H), dtype=np.float64)
    for c in range(NCORE):
        acc += res.results[c]["outp"][0:T].astype(np.float64)
        acc += res.results[c]["outsh"].astype(np.float64)
    return acc.astype(np.float32)


# revision 12
# speedup vs baseline: 1.1592x; 1.1592x over previous
"""Ernie4 MoE (T=2048, H=1024, E=64 top-6, I=512 + shared SwiGLU MLP) on 8 Trainium2 cores.

Strategy: expert parallelism, bf16 compute with an exact-fp32 router.
Each core owns 8 expert slots (host assigns experts to (core, slot) by
routed-count snake order; per-slot FFN capacities are derived from the actual
counts at first call and baked into the program). Per core:
  1. exact-fp32 gate logits (reproduces reference top-6 picks bit-exactly),
     sigmoid scores, top-6 + renormalized combine weights,
  2. per-slot routed token lists via gpsimd sparse_gather (pad slots point at
     the trash row T),
  3. transpose-gathers routed activations (bf16) straight into
     [128, H/128, slots] layout, expert SwiGLU FFN on the PE in bf16
     (fp32 PSUM) over the 16-granular capacity, scales by the fp32 combine
     weight, scatter-ADDs fp32 into outp (row T absorbs pad slots),
  4. shared MLP (SI/8 slice, bf16) written densely to a separate bf16 output.
The host sums outp[0:T] + outsh over the 8 cores in fp64.

Per-engine emission order is the core of the schedule:
  sync   : consts, gwT, xtl evens, shared W, THEN the 24MB expert-weight
           stream (sync has no compute, so HWDGE ring backpressure is free)
  scalar : xtl odds first, then sigmoids/SiLUs and the small latency-critical
           DMAs (routing roundtrip, idx replication) - never behind bulk DMA
  vector : top-k -> compaction masks -> shared-MLP tail -> expert silu tails
  PE     : gate -> shared gate/up -> counts -> shared down 0-7 -> expert FFNs
           -> shared down 8-15
  gpsimd : sparse_gather slot0 -> slot1 -> slot0 gathers -> remaining slots
           -> pipelined expert gathers/scatters
"""

import numpy as np

T, H, E, K, I, SI = 2048, 1024, 64, 6, 512, 1024
NCORE = 8
EC = E // NCORE          # expert slots per core
CMAX = 384               # hard per-slot capacity ceiling (idx tile sizing)
KC = H // 128            # hidden-dim 128-chunks
ICN = I // 128           # expert-intermediate 128-chunks
TCN = T // 128           # token 128-chunks
SIC = SI // NCORE        # shared-intermediate slice per core
BIG = 1e30

_CACHE = {}


def _rup(x, m):
    return (x + m - 1) // m * m


def _build(caps):
    """caps: per-slot FFN capacities (multiples of 16, <= CMAX)."""
    import concourse.bass as bass
    import concourse.tile as tile
    from concourse import bacc, mybir

    f32 = mybir.dt.float32
    bf16 = mybir.dt.bfloat16
    i32 = mybir.dt.int32
    i16 = mybir.dt.int16
    u32 = mybir.dt.uint32
    AF = mybir.ActivationFunctionType
    OP = mybir.AluOpType
    AX = mybir.AxisListType

    caps128 = [_rup(c, 128) for c in caps]

    nc = bacc.Bacc("TRN2", target_bir_lowering=False, debug=False,
                   enable_asserts=False, num_devices=NCORE)

    xT = nc.dram_tensor("xT", [H, T], f32, kind="ExternalInput").ap()
    xp = nc.dram_tensor("xp", [T + 1, H], bf16, kind="ExternalInput").ap()
    gwT = nc.dram_tensor("gwT", [H, E], f32, kind="ExternalInput").ap()
    biasr = nc.dram_tensor("biasr", [128, E], f32, kind="ExternalInput").ap()
    wg = nc.dram_tensor("wg", [EC, H, I], bf16, kind="ExternalInput").ap()
    wu = nc.dram_tensor("wu", [EC, H, I], bf16, kind="ExternalInput").ap()
    wd = nc.dram_tensor("wd", [EC, I, H], bf16, kind="ExternalInput").ap()
    wsg = nc.dram_tensor("wsg", [H, SIC], bf16, kind="ExternalInput").ap()
    wsu = nc.dram_tensor("wsu", [H, SIC], bf16, kind="ExternalInput").ap()
    wsd = nc.dram_tensor("wsd", [SIC, H], bf16, kind="ExternalInput").ap()
    tokp1 = nc.dram_tensor("tokp1", [16, T // 16], f32, kind="ExternalInput").ap()
    pos24 = nc.dram_tensor("pos24", [16, CMAX // 16], f32, kind="ExternalInput").ap()
    outp = nc.dram_tensor("outp", [T + 1, H], f32, kind="ExternalOutput").ap()
    outsh = nc.dram_tensor("outsh", [T, H], bf16, kind="ExternalOutput").ap()

    cmb_d = nc.dram_tensor("cmb_d", [T + 1, 64], f32, kind="Internal").ap()
    sel_d = nc.dram_tensor("sel_d", [T, EC], f32, kind="Internal").ap()

    with tile.TileContext(nc) as tc:
        with (
            tc.tile_pool(name="consts", bufs=1) as consts,
            tc.tile_pool(name="wpool", bufs=2) as wpool,
            tc.tile_pool(name="etmp", bufs=2) as etmp,
            tc.tile_pool(name="smalls", bufs=1) as smalls,
            tc.tile_pool(name="ps_small", bufs=4, space="PSUM") as ps_s,
            tc.tile_pool(name="ps_big", bufs=2, space="PSUM") as ps_b,
        ):
            # ---- consts (sync) ----
            tokp1_sb = consts.tile([16, T // 16], f32)
            nc.sync.dma_start(tokp1_sb[:], tokp1)
            bias_sb = consts.tile([128, E], f32)
            nc.sync.dma_start(bias_sb[:], biasr)
            pos_sb = consts.tile([16, CMAX // 16], f32)
            nc.sync.dma_start(pos_sb[:], pos24)
            ones128 = consts.tile([128, 1], f32)
            nc.vector.memset(ones128[:], 1.0)
            ones16 = consts.tile([1, 16], f32)
            nc.vector.memset(ones16[:], 1.0)

            idxP = smalls.tile([128, EC, CMAX // 16], i16, name="idxP")

            with (
                tc.tile_pool(name="ph1", bufs=2) as ph1,
                tc.tile_pool(name="route", bufs=1) as route,
                tc.tile_pool(name="xpool", bufs=2) as xpool,
            ):
                gwT_sb = ph1.tile([128, KC, E], f32, tag="gwT")
                nc.sync.dma_start(gwT_sb[:], gwT.rearrange("(kc p) e -> p kc e", p=128))

                # all xT slab loads issued up-front: evens on sync (before the
                # weight stream), odds on scalar (before any scalar compute)
                xtls = []
                for sl in range(8):
                    xtl = ph1.tile([128, KC, 256], f32, tag="xtl", bufs=3,
                                   name=f"xtl{sl}")
                    eng = nc.sync if sl % 2 == 0 else nc.scalar
                    eng.dma_start(
                        xtl[:], xT.rearrange("(kc p) t -> p kc t", p=128)[:, :, sl * 256:(sl + 1) * 256])
                    xtls.append(xtl)

                wsg_sb = ph1.tile([128, KC, SIC], bf16, tag="wsg")
                nc.sync.dma_start(wsg_sb[:], wsg.rearrange("(kc p) s -> p kc s", p=128))
                wsu_sb = ph1.tile([128, KC, SIC], bf16, tag="wsu")
                nc.sync.dma_start(wsu_sb[:], wsu.rearrange("(kc p) s -> p kc s", p=128))
                wsd_sb = ph1.tile([128, H], bf16, tag="wsd")
                nc.sync.dma_start(wsd_sb[:], wsd)

                # ---- expert weight stream: sync HWDGE (no compute behind it) ----
                wg_sbs, wu_sbs, wd_sbs = [], [], []
                for e in range(EC):
                    wg_sb = wpool.tile([128, KC, I], bf16, tag="wg")
                    nc.sync.dma_start(wg_sb[:], wg[e].rearrange("(kc p) i -> p kc i", p=128))
                    wu_sb = wpool.tile([128, KC, I], bf16, tag="wu")
                    nc.sync.dma_start(wu_sb[:], wu[e].rearrange("(kc p) i -> p kc i", p=128))
                    wd_sb = wpool.tile([128, ICN, H], bf16, tag="wd")
                    nc.sync.dma_start(wd_sb[:], wd[e].rearrange("(ic p) h -> p ic h", p=128))
                    wg_sbs.append(wg_sb); wu_sbs.append(wu_sb); wd_sbs.append(wd_sb)

                scores = route.tile([128, TCN, E], f32, tag="scores")

                # ---- per slab: exact-fp32 gate logits, bf16 convert, shared
                # gate/up (PE) with SiLU + copy staged to bf16 SBUF on the
                # scalar engine (PSUM freed immediately; a_s mults run later
                # on vector, after the routing-critical ops) ----
                gu_sbs, pu_sbs = [], []
                for sl in range(8):
                    xtl = xtls[sl]
                    for j in range(2):
                        tci = sl * 2 + j
                        pl = ps_s.tile([128, 512], f32, tag="mm_small")
                        for kc in range(KC):
                            nc.tensor.matmul(pl[:, :E], xtl[:, kc, j * 128:(j + 1) * 128],
                                             gwT_sb[:, kc, :], start=(kc == 0), stop=(kc == KC - 1))
                        nc.scalar.activation(scores[:, tci, :], pl[:, :E], AF.Sigmoid)
                    xbf = ph1.tile([128, KC, 256], bf16, tag="xbf", bufs=2,
                                   name=f"xbf{sl}")
                    nc.vector.tensor_copy(xbf[:], xtl[:])
                    pg = ps_s.tile([128, 512], f32, tag="mm_small")
                    pu = ps_s.tile([128, 512], f32, tag="mm_small")
                    for kc in range(KC):
                        nc.tensor.matmul(pg[:, :256], wsg_sb[:, kc, :], xbf[:, kc, :],
                                         start=(kc == 0), stop=(kc == KC - 1))
                    for kc in range(KC):
                        nc.tensor.matmul(pu[:, :256], wsu_sb[:, kc, :], xbf[:, kc, :],
                                         start=(kc == 0), stop=(kc == KC - 1))
                    sg_sb = route.tile([128, 256], f32, tag="sg_sb", bufs=8,
                                       name=f"sg{sl}")
                    nc.scalar.activation(sg_sb[:], pg[:, :256], AF.Sigmoid)
                    pg_sb = route.tile([128, 256], bf16, tag="pg_sb", bufs=8,
                                       name=f"pgs{sl}")
                    nc.scalar.activation(pg_sb[:], pg[:, :256], AF.Copy)
                    pu_sb = route.tile([128, 256], bf16, tag="pu_sb", bufs=8,
                                       name=f"pus{sl}")
                    nc.scalar.activation(pu_sb[:], pu[:, :256], AF.Copy)
                    gu_sbs.append((sg_sb, pg_sb)); pu_sbs.append(pu_sb)

                # ---- routing top-k (heads the vector queue) ----
                idxf = route.tile([16, EC, CMAX // 16], f32, tag="idxf")
                nc.vector.memset(idxf[:], 0.0)
                work_t = [route.tile([128, TCN, E], f32, tag=f"work{i}", name=f"work{i}")
                          for i in range(2)]
                nc.vector.tensor_tensor(
                    work_t[0][:], scores[:],
                    bias_sb[:, None, :].to_broadcast([128, TCN, E]), op=OP.add)
                wsrc = work_t[0]
                for k in range(K):
                    m = route.tile([128, TCN], f32, tag=f"m{k % 2}")
                    nc.vector.reduce_max(m[:], wsrc[:], axis=AX.X)
                    eq = route.tile([128, TCN, E], f32, tag="eq")
                    nc.vector.tensor_tensor(
                        eq[:], wsrc[:], m[:, :, None].to_broadcast([128, TCN, E]),
                        op=OP.is_equal)
                    wdst = work_t[(k + 1) % 2] if k < K - 1 else work_t[0]
                    nc.vector.scalar_tensor_tensor(
                        wdst[:], eq[:], -BIG, wsrc[:], op0=OP.mult, op1=OP.add)
                    wsrc = wdst
                sel = route.tile([128, TCN, E], f32, tag="eq")
                nc.vector.tensor_scalar(sel[:], wsrc[:], -BIG / 2, None, op0=OP.is_lt)
                selprod = route.tile([128, TCN, E], f32, tag="work1")
                nc.vector.tensor_tensor(selprod[:], scores[:], sel[:], op=OP.mult)
                denom = route.tile([128, TCN], f32, tag="denom")
                nc.vector.tensor_reduce(denom[:], selprod[:], axis=AX.X, op=OP.add)
                rec = route.tile([128, TCN], f32, tag="rec")
                nc.vector.reciprocal(rec[:], denom[:])
                cmb8 = route.tile([128, TCN, EC], f32, tag="cmb8")
                nc.vector.tensor_tensor(
                    cmb8[:], selprod[:, :, 0:EC],
                    rec[:, :, None].to_broadcast([128, TCN, EC]), op=OP.mult)
                cmbw = route.tile([128, TCN, 64], f32, tag="work1")
                nc.vector.memset(cmbw[:], 0.0)
                nc.vector.tensor_copy(cmbw[:, :, 0:EC], cmb8[:])
                zrow = route.tile([1, 64], f32, tag="zrow")
                nc.vector.memset(zrow[:], 0.0)

                # layout roundtrip on the scalar queue (small, latency-critical)
                nc.scalar.dma_start(
                    cmb_d[0:T].rearrange("(tc p) e -> p tc e", p=128), cmbw[:])
                nc.scalar.dma_start(cmb_d[T:T + 1, :], zrow[:])
                nc.scalar.dma_start(sel_d.rearrange("(tc p) e -> p tc e", p=128),
                                    sel[:, :, 0:EC])
                sel16 = route.tile([16, EC, T // 16], f32, tag="sel16")
                nc.scalar.dma_start(sel16[:], sel_d.rearrange("(f q) e -> q e f", q=16))

                # per-slot routed counts via PE
                pc = ps_s.tile([1, 512], f32, tag="mm_small", name="pc")
                nc.tensor.matmul(pc[0:1, 0:128], ones128[:],
                                 sel[:, :, 0:EC].rearrange("p t e -> p e t"),
                                 start=True, stop=True)
                counts = route.tile([1, EC], f32, tag="counts")
                nc.vector.tensor_reduce(counts[:], pc[0:1, 0:128].rearrange(
                    "p (e t) -> p e t", e=EC), axis=AX.X, op=OP.add)
                pnf = ps_s.tile([16, 512], f32, tag="mm_small", name="pnf")
                nc.tensor.matmul(pnf[:, 0:EC], ones16[:], counts[:],
                                 start=True, stop=True)
                nf16 = route.tile([16, EC], f32, tag="nf16")
                nc.vector.tensor_copy(nf16[:], pnf[:, 0:EC])

                # masked token values in wrapped layout: sel*(tok+1)-1
                nc.vector.tensor_tensor(
                    sel16[:], sel16[:],
                    tokp1_sb[:, None, :].to_broadcast([16, EC, T // 16]), op=OP.mult)
                nc.vector.tensor_scalar_sub(sel16[:], sel16[:], 1.0)

                # ---- compaction: slot 0 fast-pathed ----
                nfs = route.tile([1, EC], u32, tag="nfs")

                def mask_slots(elo, ehi):
                    sli = (slice(None), slice(elo, ehi), slice(None))
                    shp = [16, ehi - elo, CMAX // 16]
                    keep = route.tile([16, EC, CMAX // 16], f32, tag="keep")
                    nc.vector.tensor_tensor(
                        keep[sli], pos_sb[:, None, :].to_broadcast(shp),
                        nf16[:, elo:ehi, None].to_broadcast(shp), op=OP.is_lt)
                    k32 = route.tile([16, EC, CMAX // 16], i32, tag="k32")
                    nc.vector.tensor_copy(k32[sli], keep[sli])
                    km = route.tile([16, EC, CMAX // 16], i32, tag="km")
                    nc.vector.tensor_scalar_mul(km[sli], k32[sli], -1)
                    bits = route.tile([16, EC, CMAX // 16], i32, tag="bits")
                    nc.vector.tensor_tensor(bits[sli], idxf[sli].bitcast(i32), km[sli],
                                            op=OP.bitwise_and)
                    km1 = route.tile([16, EC, CMAX // 16], f32, tag="km1")
                    nc.vector.tensor_scalar_sub(km1[sli], keep[sli], 1.0)
                    idxnf = route.tile([16, EC, CMAX // 16], f32, tag="idxnf")
                    nc.vector.scalar_tensor_tensor(idxnf[sli], km1[sli], -float(T),
                                                   bits[sli].bitcast(f32),
                                                   op0=OP.mult, op1=OP.add)
                    nc.vector.tensor_copy(idxP[0:16, elo:ehi, :], idxnf[sli])
                    nc.scalar.dma_start(idxP[16:32, elo:ehi, :], idxP[0:16, elo:ehi, :])
                    nc.scalar.dma_start(idxP[32:64, elo:ehi, :], idxP[0:32, elo:ehi, :])
                    nc.scalar.dma_start(idxP[64:128, elo:ehi, :], idxP[0:64, elo:ehi, :])

                xgs, cgs = {}, {}

                def emit_gathers(e):
                    C1 = caps128[e]
                    xg = xpool.tile([128, KC, C1], bf16, tag=f"xe{C1}", name=f"xg{e}")
                    nc.gpsimd.dma_gather(xg[:], xp, idxP[:, e, 0:C1 // 16], C1, C1, H,
                                         transpose=True)
                    cg = xpool.tile([128, 3, 64], f32, tag="cg", name=f"cg{e}")
                    nc.gpsimd.dma_gather(cg[:, 0:C1 // 128, :], cmb_d,
                                         idxP[:, e, 0:C1 // 16], C1, C1, 64)
                    xgs[e], cgs[e] = xg, cg

                nc.gpsimd.sparse_gather(idxf[:, 0, 0:caps128[0] // 16], sel16[:, 0, :],
                                        num_found=nfs[0:1, 0:1])
                mask_slots(0, 1)
                nc.gpsimd.sparse_gather(idxf[:, 1, 0:caps128[1] // 16], sel16[:, 1, :],
                                        num_found=nfs[0:1, 1:2])
                emit_gathers(0)
                for e in range(2, EC):
                    nc.gpsimd.sparse_gather(idxf[:, e, 0:caps128[e] // 16], sel16[:, e, :],
                                            num_found=nfs[0:1, e:e + 1])
                mask_slots(1, EC)
                emit_gathers(1)

                # a_s = silu(g)*u for the shared MLP (vector, after routing ops)
                a_s = route.tile([128, 8, 256], bf16, tag="a_s")
                for sl in range(8):
                    sg_sb, pg_sb = gu_sbs[sl]
                    gu_t = route.tile([128, 256], f32, tag="gut", bufs=2)
                    nc.vector.tensor_tensor(gu_t[:], sg_sb[:], pg_sb[:], op=OP.mult)
                    nc.vector.tensor_tensor(a_s[:, sl, :], gu_t[:], pu_sbs[sl][:],
                                            op=OP.mult)

                def shared_down(tci):
                    sl, j = tci // 2, tci % 2
                    py = ps_b.tile([128, H], f32, tag="mm_big")
                    for nh in range(2):
                        nc.tensor.matmul(py[:, nh * 512:(nh + 1) * 512],
                                         a_s[:, sl, j * 128:(j + 1) * 128],
                                         wsd_sb[:, nh * 512:(nh + 1) * 512],
                                         start=True, stop=True)
                    ysh = route.tile([128, H], bf16, tag="ysh", bufs=2)
                    nc.scalar.activation(ysh[:, 0:512], py[:, 0:512], AF.Copy)
                    nc.vector.tensor_copy(ysh[:, 512:1024], py[:, 512:1024])
                    nc.scalar.dma_start(
                        outsh.rearrange("(tc p) h -> p tc h", p=128)[:, tci, :], ysh[:])

                # first half of the shared down-proj fills the PE gap while
                # slot-0's gather completes
                for tci in range(8):
                    shared_down(tci)

                # ---- expert loop ----
                for e in range(EC):
                    C, C1 = caps[e], caps128[e]
                    CCH = (C + 127) // 128
                    if e + 1 < EC:
                        emit_gathers(e + 1)
                    wg_sb, wu_sb, wd_sb = wg_sbs[e], wu_sbs[e], wd_sbs[e]
                    xeT, cg = xgs.pop(e), cgs.pop(e)

                    aT = xpool.tile([128, ICN, CMAX], bf16, tag="aT")
                    for ic in range(ICN):
                        pg = ps_s.tile([128, 512], f32, tag="mm_small")
                        pu = ps_s.tile([128, 512], f32, tag="mm_small")
                        for kc in range(KC):
                            nc.tensor.matmul(pg[:, :C], wg_sb[:, kc, ic * 128:(ic + 1) * 128],
                                             xeT[:, kc, 0:C], start=(kc == 0), stop=(kc == KC - 1))
                        for kc in range(KC):
                            nc.tensor.matmul(pu[:, :C], wu_sb[:, kc, ic * 128:(ic + 1) * 128],
                                             xeT[:, kc, 0:C], start=(kc == 0), stop=(kc == KC - 1))
                        sg_t = etmp.tile([128, CMAX], f32, tag="esilu")
                        nc.scalar.activation(sg_t[:, :C], pg[:, :C], AF.Sigmoid)
                        gu_t = etmp.tile([128, CMAX], f32, tag="egu")
                        nc.vector.tensor_tensor(gu_t[:, :C], sg_t[:, :C], pg[:, :C], op=OP.mult)
                        nc.vector.tensor_tensor(aT[:, ic, 0:C], gu_t[:, :C], pu[:, :C], op=OP.mult)

                    y_sb = xpool.tile([128, 3, H], f32, tag="y")
                    for cc in range(CCH):
                        w = min(128, C - cc * 128)
                        if w < 128:
                            nc.vector.memset(y_sb[:, cc, :], 0.0)
                        py = ps_b.tile([128, H], f32, tag="mm_big")
                        for ic in range(ICN):
                            for nh in range(2):
                                nc.tensor.matmul(py[0:w, nh * 512:(nh + 1) * 512],
                                                 aT[:, ic, cc * 128:cc * 128 + w],
                                                 wd_sb[:, ic, nh * 512:(nh + 1) * 512],
                                                 start=(ic == 0), stop=(ic == ICN - 1))
                        nc.scalar.activation(y_sb[0:w, cc, :], py[0:w, :], AF.Copy,
                                             scale=cg[0:w, cc, e:e + 1])
                    nc.gpsimd.dma_scatter_add(outp, y_sb[:, 0:CCH, :],
                                              idxP[:, e, 0:C // 16], C, C, H)

                # remaining shared down-proj chunks
                for tci in range(8, TCN):
                    shared_down(tci)

    nc.compile()
    return nc


def _route_counts(x, gate_w, gate_bias):
    """Host-side routing counts (fp64) for load-balanced expert assignment."""
    logits = x.astype(np.float64) @ gate_w.astype(np.float64).T
    scores = 1.0 / (1.0 + np.exp(-logits))
    idx = np.argsort(-(scores + gate_bias.astype(np.float64)), axis=1)[:, :K]
    return np.bincount(idx.ravel(), minlength=E)


def _assign(counts):
    """Snake expert->(core,slot) assignment + 16-granular slot capacities."""
    order = np.argsort(-counts, kind="stable")
    perm = np.zeros((NCORE, EC), dtype=np.int64)
    caps = []
    for s in range(EC):
        band = order[8 * s:8 * s + 8]
        perm[:, s] = band if s % 2 == 0 else band[::-1]
        c = _rup(int(counts[band].max()) + 8, 16)
        assert c <= CMAX, (s, c)
        caps.append(c)
    return perm, tuple(caps)


def _prep_in_maps(inputs, perm):
    import ml_dtypes
    bf = ml_dtypes.bfloat16
    x = np.ascontiguousarray(inputs["hidden_states"], dtype=np.float32)
    gate_w = np.asarray(inputs["gate_w"], dtype=np.float32)
    gate_bias = np.asarray(inputs["gate_bias"], dtype=np.float32)
    w_gate = np.asarray(inputs["w_gate"], dtype=np.float32)
    w_up = np.asarray(inputs["w_up"], dtype=np.float32)
    w_down = np.asarray(inputs["w_down"], dtype=np.float32)
    ws_gate = np.asarray(inputs["ws_gate"], dtype=np.float32)
    ws_up = np.asarray(inputs["ws_up"], dtype=np.float32)
    ws_down = np.asarray(inputs["ws_down"], dtype=np.float32)

    xTc = np.ascontiguousarray(x.T)
    xbf = np.vstack([x, np.zeros((1, H), np.float32)]).astype(bf)
    tokp1 = (np.arange(16)[:, None] + 16 * np.arange(T // 16)[None, :] + 1).astype(np.float32)
    pos24 = (np.arange(16)[:, None] + 16 * np.arange(CMAX // 16)[None, :]).astype(np.float32)

    in_maps = []
    for c in range(NCORE):
        loc = list(perm[c])
        gorder = loc + [e for e in range(E) if e not in loc]
        in_maps.append({
            "xp": xbf,
            "xT": xTc,
            "gwT": np.ascontiguousarray(gate_w[gorder].T),
            "biasr": np.ascontiguousarray(
                np.broadcast_to(gate_bias[0, gorder], (128, E))).astype(np.float32),
            "wg": np.ascontiguousarray(w_gate[loc]).astype(bf),
            "wu": np.ascontiguousarray(w_up[loc]).astype(bf),
            "wd": np.ascontiguousarray(w_down[loc]).astype(bf),
            "wsg": np.ascontiguousarray(ws_gate[:, c * SIC:(c + 1) * SIC]).astype(bf),
            "wsu": np.ascontiguousarray(ws_up[:, c * SIC:(c + 1) * SIC]).astype(bf),
            "wsd": np.ascontiguousarray(ws_down[c * SIC:(c + 1) * SIC, :]).astype(bf),
            "tokp1": tokp1,
            "pos24": pos24,
        })
    return in_maps


def get_nc(inputs):
    counts = _route_counts(inputs["hidden_states"], inputs["gate_w"], inputs["gate_bias"])
    perm, caps = _assign(counts)
    if ("nc", caps) not in _CACHE:
        _CACHE[("nc", caps)] = _build(caps)
    return _CACHE[("nc", caps)], perm


def kernel(**inputs) -> np.ndarray:
    from concourse import bass_utils
    nc, perm = get_nc(inputs)
    in_maps = _prep_in_maps(inputs, perm)
    res = bass_utils.run_bass_kernel_spmd(nc, in_maps, core_ids=list(range(NCORE)))
    acc = np.zeros((T, H), dtype=np.float64)
    for c in range(NCORE):
        acc += res.results[c]["outp"][0:T].astype(np.float64)
        acc += res.results[c]["outsh"].astype(np.float64)
    return acc.astype(np.float32)
